# revision 10
# baseline (speedup 1.0000x reference)
"""Trainium2 Bass kernel for the CrossAttention_MP problem.

Self-contained: hardcodes shapes B=8, C=512, H=W=96. Shards batch over the
8 NeuronCores (data-parallel, no collectives). See NOTES.md for the index
algebra that lets the torch-faithful qkv reshape collapse into strided APs.
"""

from contextlib import ExitStack

import numpy as np

import concourse.bass as bass
import concourse.mybir as mybir
import concourse.tile as tile
from concourse import bacc
from concourse.bass_utils import run_bass_kernel_spmd

B, C, H, W = 8, 512, 96, 96
HW = H * W            # 9216
P3 = HW // 3          # 3072 spatial positions per third
O3 = 3 * C            # 1536 qkv output channels
BLK_C = 64            # channels per block
NBLK = C // BLK_C     # 8
BP = 6 * BLK_C        # 384 p-columns per block

F32 = mybir.dt.float32
F32R = mybir.dt.float32r
BF16 = mybir.dt.bfloat16
EXP = mybir.ActivationFunctionType.Exp

_CACHE = {}


def _build_kernel_body(tc, out, yq, xk, xv, wt, n_blocks=NBLK, es=None):
    nc = tc.nc

    const = es.enter_context(tc.tile_pool(name="const", bufs=1))
    inpool = es.enter_context(tc.tile_pool(name="inp", bufs=2))
    qkpool = es.enter_context(tc.tile_pool(name="qk", bufs=2))
    mvpool = es.enter_context(tc.tile_pool(name="mv", bufs=2))
    vspool = es.enter_context(tc.tile_pool(name="vs", bufs=2))
    epool = es.enter_context(tc.tile_pool(name="ep", bufs=4))
    ospool = es.enter_context(tc.tile_pool(name="os", bufs=4))
    rzpool = es.enter_context(tc.tile_pool(name="rz", bufs=4))
    vdpool = es.enter_context(tc.tile_pool(name="vd", bufs=2, space="DRAM"))
    ppsum = es.enter_context(tc.tile_pool(name="ppsum", bufs=2, space="PSUM"))
    spsum = es.enter_context(tc.tile_pool(name="spsum", bufs=3, space="PSUM"))
    opsum = es.enter_context(tc.tile_pool(name="opsum", bufs=3, space="PSUM"))

    # W^T resident: [128 cin_part, 4 cin_chunk, 1536 o]
    wtt = const.tile([128, 4, O3], F32R)
    for kc in range(4):
        nc.sync.dma_start(out=wtt[:, kc, :], in_=wt[128 * kc : 128 * (kc + 1), :])

    for blk in range(n_blocks):
        c0 = blk * BLK_C
        p0 = 6 * c0

        # ---- load input slices [512 cin, BP p] as [128, 4, BP]
        yt = inpool.tile([128, 4, BP], F32R, tag="yt")
        xkt = inpool.tile([128, 4, BP], F32R, tag="xkt")
        xvt = inpool.tile([128, 4, BP], F32R, tag="xvt")
        for src_d, dst_t in ((yq, yt), (xk, xkt), (xv, xvt)):
            for kc in range(4):
                nc.sync.dma_start(
                    out=dst_t[:, kc, :],
                    in_=src_d[128 * kc : 128 * (kc + 1), p0 : p0 + BP],
                )

        # ---- q/k projections, M^T layout in o-chunks of 96.
        # PSUM tile for o-chunk r = [96 j, 64 c, 6 q]; the copy interleaves it
        # into per-channel transposed grids: QT[j, c, i = 16q + r] so each
        # channel's lhsT/rhs is a contiguous [96, 96] slice.
        qt = qkpool.tile([96, BLK_C, 96], BF16, tag="qt")
        kt = qkpool.tile([96, BLK_C, 96], BF16, tag="kt")
        for src_t, dst_t in ((yt, qt), (xkt, kt)):
            dst_v = dst_t.rearrange("j c (q r2) -> j r2 c q", r2=16)
            for r in range(16):
                ps = ppsum.tile([96, 512], F32, tag="pp")
                for kc in range(4):
                    nc.tensor.matmul(
                        ps[:, 0:BP],
                        lhsT=wtt[:, kc, 96 * r : 96 * (r + 1)],
                        rhs=src_t[:, kc, :],
                        start=(kc == 0),
                        stop=(kc == 3),
                    )
                nc.vector.tensor_copy(
                    dst_v[:, r], ps[:, 0:BP].rearrange("j (c q) -> j c q", q=6)
                )

        # ---- v projection, M layout in p-chunks of 96:
        # Mv[p_local % 96, pc, o] = M[p0 + 96*pc + p_local%96, o]
        mv = mvpool.tile([96, 4, O3], BF16, tag="mv")
        for pc in range(4):
            for oc in range(3):
                ps = ppsum.tile([96, 512], F32, tag="pp")
                for kc in range(4):
                    nc.tensor.matmul(
                        ps[:],
                        lhsT=xvt[:, kc, 96 * pc : 96 * (pc + 1)],
                        rhs=wtt[:, kc, 512 * oc : 512 * (oc + 1)],
                        start=(kc == 0),
                        stop=(kc == 3),
                    )
                nc.scalar.copy(mv[:, pc, 512 * oc : 512 * (oc + 1)], ps[:])

        # ---- assemble V tiles for the whole block via a DRAM bounce:
        # Vs[k = 16q + r, c_local, w] = V_c[k, w] = M[p_local = 6*c_local + q, 96r + w]
        vd = vdpool.tile([4 * 96, O3], BF16, tag="vd")  # [384 p_local, 1536 o]
        for pc in range(4):
            nc.sync.dma_start(out=vd[96 * pc : 96 * (pc + 1), :], in_=mv[:, pc, :])
        vs = vspool.tile([96, BLK_C, 97], BF16, tag="vs")
        nc.vector.memset(vs[:, :, 96], 1.0)  # ones column for the Z row-sums
        vd_v = vd.rearrange("(c q) (r w) -> q r c w", q=6, r=16)
        for q in range(6):
            nc.sync.dma_start(out=vs[16 * q : 16 * (q + 1), :, 0:96], in_=vd_v[q])

        # ---- per-channel attention
        for cl in range(BLK_C):
            sp = spsum.tile([96, 96], F32, tag="sp")
            nc.tensor.matmul(
                sp[:], lhsT=kt[:, cl, :], rhs=qt[:, cl, :], start=True, stop=True
            )

            e = epool.tile([96, 96], BF16, tag="e")
            nc.scalar.activation(e[:], sp[:], EXP, scale=0.125)

            op = opsum.tile([96, 97], F32, tag="op")
            nc.tensor.matmul(op[:], lhsT=e[:], rhs=vs[:, cl, :], start=True, stop=True)

            rz = rzpool.tile([96, 1], F32, tag="rz")
            nc.vector.reciprocal(rz[:], op[:, 96:97])
            osb = ospool.tile([96, 96], F32, tag="os")
            nc.vector.tensor_scalar_mul(osb[:], op[:, 0:96], rz[:])
            nc.sync.dma_start(out=out[c0 + cl], in_=osb[:])


def build_program(n_blocks=NBLK):
    key = ("prog", n_blocks)
    if key in _CACHE:
        return _CACHE[key]
    nc = bacc.Bacc("TRN2", target_bir_lowering=False, debug=False)
    yq = nc.dram_tensor("yq", [C, P3], F32R, kind="ExternalInput").ap()
    xk = nc.dram_tensor("xk", [C, P3], F32R, kind="ExternalInput").ap()
    xv = nc.dram_tensor("xv", [C, P3], F32R, kind="ExternalInput").ap()
    wt = nc.dram_tensor("wt", [C, O3], F32R, kind="ExternalInput").ap()
    out = nc.dram_tensor("out", [C, H, W], F32, kind="ExternalOutput").ap()
    with tile.TileContext(nc) as tc, ExitStack() as es:
        _build_kernel_body(tc, out, yq, xk, xv, wt, n_blocks=n_blocks, es=es)
    nc.compile()
    _CACHE[key] = nc
    return nc


def make_in_maps(x, y, W_qkv):
    """Host-side sharding + layout prep (batch-parallel over 8 cores)."""
    x = np.asarray(x, dtype=np.float32)
    y = np.asarray(y, dtype=np.float32)
    wt_full = np.ascontiguousarray(np.asarray(W_qkv, dtype=np.float32).T)  # [512,1536]
    in_maps = []
    for b in range(B):
        in_maps.append(
            {
                "yq": np.ascontiguousarray(y[b, :, 0:32, :].reshape(C, P3)),
                "xk": np.ascontiguousarray(x[b, :, 32:64, :].reshape(C, P3)),
                "xv": np.ascontiguousarray(x[b, :, 64:96, :].reshape(C, P3)),
                "wt": wt_full,
            }
        )
    return in_maps


def kernel(x, y, W_qkv, _trace=False):
    nc = build_program()
    in_maps = make_in_maps(x, y, W_qkv)
    res = run_bass_kernel_spmd(nc, in_maps, core_ids=list(range(B)), trace=_trace)
    out = np.stack([res.results[b]["out"] for b in range(B)]).astype(np.float32)
    if _trace:
        kernel.last_results = res
    return out
